# revision 4
# baseline (speedup 1.0000x reference)
"""MHA (ALiBi + causal) Trainium2 Bass kernel, 8-core SPMD.

Sharding: 8 cores = 2 (batch) x 4 (head groups of 4 heads).
Each core computes, for its batch element and its 4 heads:
  qkv projection -> causal attention (unnormalized probs, denom folded
  into the y-scale) -> partial O-projection (its rows of W_o).
Host sums the 4 partial [S,D] outputs per batch element.

All heavy matmuls run in bf16 with fp32 PSUM accumulation; softmax is
fp32. ALiBi slope*j is added exactly (fp32) by fusing it into the
PSUM->SBUF score copy (DVE tensor_add with a host-precomputed
broadcast row). The per-row -slope*i term cancels in softmax.
"""

import math
import sys
from contextlib import ExitStack

import numpy as np

sys.path.insert(0, "/opt/trn_rl_repo")

import ml_dtypes  # noqa: E402

BF16 = ml_dtypes.bfloat16

# Problem constants (hardcoded per contract)
B = 2
S = 2048
D = 2048
NHEAD = 16
HD = 128
NCORES = 8
GROUPS = 4          # head groups (tensor-parallel)
NH = NHEAD // GROUPS  # heads per core = 4
P = 128
DC = D // P         # 16 contraction chunks
ST = S // P         # 16 s-tiles
NEG = -1.0e30

_CACHE = {}


def _build():
    import concourse.bass as bass
    import concourse.mybir as mybir
    import concourse.tile as tile
    from concourse import bacc
    from concourse.masks import make_identity

    f32 = mybir.dt.float32
    bf16 = mybir.dt.bfloat16

    nc = bacc.Bacc("TRN2", target_bir_lowering=False, debug=False)

    xt = nc.dram_tensor("xt", [D, S], bf16, kind="ExternalInput").ap()
    wq = nc.dram_tensor("wq", [D, NH * HD], bf16, kind="ExternalInput").ap()
    wk = nc.dram_tensor("wk", [D, NH * HD], bf16, kind="ExternalInput").ap()
    wv = nc.dram_tensor("wv", [D, NH * HD], bf16, kind="ExternalInput").ap()
    wo = nc.dram_tensor("wo", [NH * HD, D], bf16, kind="ExternalInput").ap()
    arow = nc.dram_tensor("arow", [NH, P, S], f32, kind="ExternalInput").ap()
    maskd = nc.dram_tensor("maskd", [P, P], f32, kind="ExternalInput").ap()
    out = nc.dram_tensor("out", [S, D], f32, kind="ExternalOutput").ap()

    with tile.TileContext(nc) as tc, ExitStack() as ctx:
        const = ctx.enter_context(tc.tile_pool(name="const", bufs=1))
        xtp = ctx.enter_context(tc.tile_pool(name="xt", bufs=1))
        whp = ctx.enter_context(tc.tile_pool(name="wh", bufs=2))
        qkp = ctx.enter_context(tc.tile_pool(name="qkT", bufs=1))
        vp = ctx.enter_context(tc.tile_pool(name="v", bufs=1))
        arp = ctx.enter_context(tc.tile_pool(name="ar", bufs=2))
        ssp = ctx.enter_context(tc.tile_pool(name="ss", bufs=2))
        pp = ctx.enter_context(tc.tile_pool(name="pb", bufs=2))
        ptp = ctx.enter_context(tc.tile_pool(name="pt", bufs=2))
        ytp = ctx.enter_context(tc.tile_pool(name="yT", bufs=1))
        osb = ctx.enter_context(tc.tile_pool(name="osb", bufs=2))
        sml = ctx.enter_context(tc.tile_pool(name="small", bufs=4))
        ybp = ctx.enter_context(tc.tile_pool(name="yblk", bufs=3))
        ps512 = ctx.enter_context(tc.tile_pool(name="ps512", bufs=4, space="PSUM"))
        ps128 = ctx.enter_context(tc.tile_pool(name="ps128", bufs=2, space="PSUM"))

        ident = const.tile([P, P], bf16, tag="ident")
        make_identity(nc, ident)
        maskt = const.tile([P, P], f32, tag="maskt")
        nc.sync.dma_start(out=maskt[:, :], in_=maskd)
        wv_sb = const.tile([P, DC, NH * HD], bf16, tag="wv")
        nc.sync.dma_start(out=wv_sb[:, :, :], in_=wv.rearrange("(c p) n -> p c n", p=P))
        wo_sb = const.tile([P, NH, D], bf16, tag="wo")
        nc.sync.dma_start(out=wo_sb[:, :, :], in_=wo.rearrange("(h p) n -> p h n", p=P))
        rden_all = const.tile([P, NH * ST], f32, tag="rden")

        qkT = [qkp.tile([P, S], bf16, tag=f"qkT{i}", name=f"qkT{i}") for i in range(2 * NH)]
        v_sb = vp.tile([P, ST, NH * HD], bf16, tag="v")
        yT = ytp.tile([P, NH * ST * P], bf16, tag="yT")

        # ---- Phase 1: projections (two S-halves to bound xt residency) ----
        HS = S // 2  # 1024
        HN = HS // 512  # 2 n-chunks per half
        for ha in range(2):
            xt_t = xtp.tile([P, DC, HS], bf16, tag="xt")
            nc.sync.dma_start(
                out=xt_t[:, :, :],
                in_=xt[:, ha * HS:(ha + 1) * HS].rearrange("(c p) s -> p c s", p=P),
            )
            # q/k: out tile (head i, col range) accumulated over DC chunks
            for i in range(2 * NH):
                w_dram = wq if i < NH else wk
                h = i % NH
                wh_t = whp.tile([P, DC, P], bf16, tag="wh")
                nc.sync.dma_start(
                    out=wh_t[:, :, :],
                    in_=w_dram[:, h * HD:(h + 1) * HD].rearrange(
                        "(c p) m -> p c m", p=P
                    ),
                )
                for n in range(HN):
                    ps = ps512.tile([P, 512], mybir.dt.float32, tag="mm512")
                    for dc in range(DC):
                        nc.tensor.matmul(
                            ps[:, :],
                            wh_t[:, dc, :],
                            xt_t[:, dc, n * 512:(n + 1) * 512],
                            start=(dc == 0),
                            stop=(dc == DC - 1),
                        )
                    nc.vector.tensor_copy(
                        qkT[i][:, ha * HS + n * 512: ha * HS + (n + 1) * 512],
                        ps[:, :],
                    )
            # v: s-tiles in this half
            for st in range(ha * (ST // 2), (ha + 1) * (ST // 2)):
                ps = ps512.tile([P, 512], mybir.dt.float32, tag="mm512")
                lo = st * P - ha * HS
                for dc in range(DC):
                    nc.tensor.matmul(
                        ps[:, :],
                        xt_t[:, dc, lo:lo + P],
                        wv_sb[:, dc, :],
                        start=(dc == 0),
                        stop=(dc == DC - 1),
                    )
                nc.vector.tensor_copy(v_sb[:, st, :], ps[:, :])

        # ---- Phase 2: attention per (head, q-tile) ----
        for h in range(NH):
            ar_t = arp.tile([P, S], mybir.dt.float32, tag="ar")
            nc.sync.dma_start(out=ar_t[:, :], in_=arow[h, :, :])
            for qi in range(ST):
                L = (qi + 1) * P
                s_sb = ssp.tile([P, S], mybir.dt.float32, tag="ss")
                c = 0
                while c * 512 < L:
                    lc = min(512, L - c * 512)
                    ps = ps512.tile([P, 512], mybir.dt.float32, tag="mm512")
                    nc.tensor.matmul(
                        ps[:, :lc],
                        qkT[h][:, qi * P:(qi + 1) * P],
                        qkT[NH + h][:, c * 512:c * 512 + lc],
                        start=True,
                        stop=True,
                    )
                    # fused psum->sbuf copy + exact fp32 ALiBi row add
                    nc.vector.tensor_add(
                        s_sb[:, c * 512:c * 512 + lc],
                        ps[:, :lc],
                        ar_t[:, c * 512:c * 512 + lc],
                    )
                    c += 1
                # causal mask inside the diagonal block
                nc.vector.tensor_add(
                    s_sb[:, L - P:L], s_sb[:, L - P:L], maskt[:, :]
                )
                mx = sml.tile([P, 1], mybir.dt.float32, tag="mx")
                nc.vector.reduce_max(
                    mx[:, :], s_sb[:, :L], axis=mybir.AxisListType.X
                )
                nmx = sml.tile([P, 1], mybir.dt.float32, tag="nmx")
                nc.vector.tensor_scalar_mul(nmx[:, :], mx[:, :], -1.0)
                p_sb = pp.tile([P, S], bf16, tag="pb")
                den = sml.tile([P, 1], mybir.dt.float32, tag="den")
                nc.scalar.activation(
                    p_sb[:, :L],
                    s_sb[:, :L],
                    mybir.ActivationFunctionType.Exp,
                    bias=nmx[:, :],
                    scale=1.0,
                    accum_out=den[:, :],
                )
                nc.vector.reciprocal(
                    rden_all[:, h * ST + qi: h * ST + qi + 1], den[:, :]
                )
                pT = ptp.tile([P, S], bf16, tag="pt")
                for kt in range(qi + 1):
                    tp = ps128.tile([P, P], bf16, tag="tp128")
                    nc.tensor.transpose(
                        tp[:, :], p_sb[:, kt * P:(kt + 1) * P], ident[:, :]
                    )
                    nc.vector.tensor_copy(pT[:, kt * P:(kt + 1) * P], tp[:, :])
                y_ps = ps128.tile([P, P], mybir.dt.float32, tag="mm128")
                for kt in range(qi + 1):
                    nc.tensor.matmul(
                        y_ps[:, :],
                        pT[:, kt * P:(kt + 1) * P],
                        v_sb[:, kt, h * HD:(h + 1) * HD],
                        start=(kt == 0),
                        stop=(kt == qi),
                    )
                y_blk = ybp.tile([P, P], bf16, tag="yblk")
                nc.scalar.activation(
                    y_blk[:, :],
                    y_ps[:, :],
                    mybir.ActivationFunctionType.Copy,
                    scale=rden_all[:, h * ST + qi: h * ST + qi + 1],
                )
                yt_ps = ps128.tile([P, P], bf16, tag="tp128")
                nc.tensor.transpose(yt_ps[:, :], y_blk[:, :], ident[:, :])
                nc.vector.tensor_copy(
                    yT[:, (h * ST + qi) * P:(h * ST + qi + 1) * P], yt_ps[:, :]
                )

        # ---- Phase 3: partial O-projection ----
        for st in range(ST):
            o_sb = osb.tile([P, D], mybir.dt.float32, tag="osb")
            for n in range(D // 512):
                ps = ps512.tile([P, 512], mybir.dt.float32, tag="mm512")
                for h in range(NH):
                    nc.tensor.matmul(
                        ps[:, :],
                        yT[:, (h * ST + st) * P:(h * ST + st + 1) * P],
                        wo_sb[:, h, n * 512:(n + 1) * 512],
                        start=(h == 0),
                        stop=(h == NH - 1),
                    )
                nc.vector.tensor_copy(o_sb[:, n * 512:(n + 1) * 512], ps[:, :])
            nc.sync.dma_start(
                out=out[st * P:(st + 1) * P, :], in_=o_sb[:, :]
            )

    nc.compile()
    return nc


def _prep_inputs(x, W_qkv, W_o):
    """Build the 8 per-core input maps."""
    scale = 1.0 / math.sqrt(HD)
    slopes = 2.0 ** (-8.0 / np.arange(1, NHEAD + 1, dtype=np.float32))
    j = np.arange(S, dtype=np.float32)
    ii = np.arange(P)[:, None]
    jj = np.arange(P)[None, :]
    maskd = np.where(jj > ii, np.float32(NEG), np.float32(0.0)).astype(np.float32)

    in_maps = []
    for c in range(NCORES):
        b, g = divmod(c, GROUPS)
        hs = g * NH * HD
        he = hs + NH * HD
        xt = np.ascontiguousarray(x[b].T).astype(BF16)
        wq = (W_qkv[:, hs:he] * scale).astype(BF16)
        wk = W_qkv[:, D + hs:D + he].astype(BF16)
        wv = W_qkv[:, 2 * D + hs:2 * D + he].astype(BF16)
        wo = np.ascontiguousarray(W_o[hs:he, :]).astype(BF16)
        sl = slopes[g * NH:(g + 1) * NH]  # [4]
        ar = np.broadcast_to(
            (sl[:, None] * j[None, :])[:, None, :], (NH, P, S)
        ).astype(np.float32)
        in_maps.append(
            {
                "xt": xt,
                "wq": wq,
                "wk": wk,
                "wv": wv,
                "wo": np.ascontiguousarray(wo),
                "arow": np.ascontiguousarray(ar),
                "maskd": maskd,
            }
        )
    return in_maps


def kernel(x, W_qkv, W_o, _trace=False):
    x = np.asarray(x, dtype=np.float32)
    W_qkv = np.asarray(W_qkv, dtype=np.float32)
    W_o = np.asarray(W_o, dtype=np.float32)

    if "nc" not in _CACHE:
        _CACHE["nc"] = _build()
    nc = _CACHE["nc"]

    from concourse import bass_utils

    in_maps = _prep_inputs(x, W_qkv, W_o)
    res = bass_utils.run_bass_kernel_spmd(
        nc, in_maps, core_ids=list(range(NCORES)), trace=_trace
    )
    _CACHE["last_result"] = res
    outs = [r["out"].astype(np.float32) for r in res.results]
    full = np.empty((B, S, D), dtype=np.float32)
    for b in range(B):
        full[b] = outs[4 * b] + outs[4 * b + 1] + outs[4 * b + 2] + outs[4 * b + 3]
    return full
